# revision 14
# baseline (speedup 1.0000x reference)
"""CE + CJS loss kernel for Trainium2, data-parallel over 8 NeuronCores.

Math (reference):
    logp = log_softmax(pred_logit, axis=1)          # xp = x - lse
    ce   = -mean_i( sum_j gt*logp )
    p    = softmax(pred_logit)
    m    = 0.5*(gt + p + EPS)
    contrib = gt*ln(gt) + p*xp - (gt+p)*ln(m)       # per element
    loss = ce + 0.25 * sum_j w_j * colsum_j(contrib) / B,   w_j = C - j

Kernel decomposition (K-shift, low cancellation):
    Because gt + p - u == 0 (u = gt+p), shifting every log-factor by a
    constant K leaves contrib unchanged:
        contrib = gt*(lng+K) + p*(xp+K) - u*(logm+K)
    The shift is free: Ln(e^K * v) = ln(v) + K folds into the ScalarE
    activation's scale. With K ~ -E[logm] all product factors are O(1)
    instead of O(-9.5), so the bf16 products lose the 50x cancellation
    blow-up (sim rel err 3e-5 vs 6e-3 for the unshifted form).

    W_j  = colsum( gt*lngK + p*x - u*logmK ) + (K-lse)-weighted colsum(p)
           (PSUM set 1; the (K-lse)*p term is a matmul with lhs=(K-lse),
            making xp available without a VectorE pass)
    CE_j = colsum( gt*x );  ce_total = sum_j CE_j - sum_i lse_i
           (uses rowsum(gt)=1 from the reference's normalization)
    Host applies w_j and assembles the loss in float64.

Engine balance per core (HBM roofline ~94us):
    ScalarE: Exp(x)+rowsum, Ln(e^K g), Ln(.5e^K u + .5e^K eps)   3 passes
    VectorE: p (4x ts), u, P1=g*lngK, P2=u*logmK, P3=p*x, P4=g*x (2x tt)
    TensorE: 5 colsum matmul streams, 80 matmuls per block
    GpSimd:  casting input DMAs only
"""
import numpy as np

import concourse.bass as bass
import concourse.tile as tile
from concourse import mybir
from concourse.bass_utils import run_bass_kernel_spmd
from concourse.vector_clock import ScopedClock

B, C = 4096, 8192
N_CORES = 8
ROWS = B // N_CORES          # 512 rows per core
N_BLK = ROWS // 128          # 4 partition blocks
F2 = 4096                    # half-row chunk
N_SLICE = C // 512           # 16 column slices per block
EPS = 1e-8
KSH = 9.3                    # log-shift constant
EK = float(np.exp(KSH))

f32 = mybir.dt.float32
bf16 = mybir.dt.bfloat16
AF = mybir.ActivationFunctionType
ALU = mybir.AluOpType


def _patched_drain_and_barrier(self, tick_clock, wait_clock):
    # Walrus CoreV3 codegen allows only ONE sync-wait command on a
    # Drain/NoOp (NO_STRUCT ctrl). The stock Tile tail drain carries one
    # wait per pending engine clock and fails to compile. Split the waits
    # across single-wait SP nops; SP executes in program order, so the
    # drain still orders after everything.
    nc = self.nc
    probe = nc.sync.nop().ins
    wait_clock.add_sem_waits(probe, ScopedClock({None: tick_clock.global_clock}))
    waits = list(probe.sync_info.on_wait) if probe.sync_info else []
    probe.sync_info = mybir.SyncInfo(on_wait=waits[:1], on_update=[])
    for w in waits[1:]:
        extra = nc.sync.nop().ins
        extra.sync_info = mybir.SyncInfo(on_wait=[w], on_update=[])
    nc.sync.drain()
    nc.all_engine_barrier()
    assert self.sems is not None
    popped = nc._tile_sem_poison_stack.pop()
    assert popped is self._sem_poison
    nc.clear_and_free_semaphores(list(self.sems.allocated().values()))
    # No trailing all_engine_barrier: the program ends here, NEFF
    # completion already waits for every queue to drain, and the sem
    # clears sit in the GpSimd queue behind the barrier above - they
    # retire before completion regardless.


tile.TileContext._drain_and_barrier = _patched_drain_and_barrier


def _split_excess_waits(nc: bass.Bass, max_waits: int = 1):
    # Same walrus limitation, general form: cap sync waits per instruction,
    # hoisting the excess onto same-engine NOPs inserted just before (the
    # engine executes its stream in order, so semantics are unchanged).
    for bb in nc.main_func.blocks:
        insts = list(bb.instructions)
        out, changed = [], False
        for ins in insts:
            si = ins.sync_info
            waits = list(si.on_wait) if (si is not None and si.on_wait) else []
            if len(waits) > max_waits:
                ups = list(si.on_update) if si.on_update else []
                for w in waits[:-max_waits]:
                    nop = mybir.InstNoOp(
                        name=nc.get_next_instruction_name(), ins=[], outs=[])
                    nop.engine = ins.engine
                    nop.sync_info = mybir.SyncInfo(on_wait=[w], on_update=[])
                    nc.register_instruction(nop)
                    out.append(nop)
                ins.sync_info = mybir.SyncInfo(
                    on_wait=waits[-max_waits:], on_update=ups)
                changed = True
            out.append(ins)
        if changed:
            bb.instructions = out


def build_nc() -> bass.Bass:
    nc = bass.Bass()
    x_dram = nc.declare_dram_parameter("pred_logit", [ROWS, C], f32, isOutput=False)
    gt_dram = nc.declare_dram_parameter("gt", [ROWS, C], f32, isOutput=False)
    pw_dram = nc.declare_dram_parameter("partials_w", [N_SLICE, 512], f32, isOutput=True)
    pce_dram = nc.declare_dram_parameter("partials_ce", [N_SLICE, 512], f32, isOutput=True)
    lse_dram = nc.declare_dram_parameter("lse_out", [128, N_BLK], f32, isOutput=True)

    from contextlib import ExitStack
    with tile.TileContext(nc) as tc, ExitStack() as es:
        consts = es.enter_context(tc.tile_pool(name="consts", bufs=1))
        xpool = es.enter_context(tc.tile_pool(name="xpool", bufs=2))
        gpool = es.enter_context(tc.tile_pool(name="gpool", bufs=2))
        tp = es.enter_context(tc.tile_pool(name="tp", bufs=1))
        pp = es.enter_context(tc.tile_pool(name="pp", bufs=1))
        up = es.enter_context(tc.tile_pool(name="up", bufs=1))
        lngp = es.enter_context(tc.tile_pool(name="lngp", bufs=1))
        logmp = es.enter_context(tc.tile_pool(name="logmp", bufs=1))
        prod = es.enter_context(tc.tile_pool(name="prod", bufs=3))
        rowp = es.enter_context(tc.tile_pool(name="rowp", bufs=2))
        psum = es.enter_context(tc.tile_pool(name="psum", bufs=1, space="PSUM"))

        ones = consts.tile([128, 1], bf16)
        nc.vector.memset(ones, 1.0)
        negones = consts.tile([128, 1], bf16)
        nc.vector.memset(negones, -1.0)
        eps_b = consts.tile([128, 1], f32)
        nc.vector.memset(eps_b, 0.5 * EK * EPS)
        lse_keep = consts.tile([128, N_BLK], f32)

        # PSUM: W set (banks 0-3) accumulates g*lngK + p*x - u*logmK
        # + (K-lse)*p; CE set (banks 4-7) accumulates g*x. 4 slices per
        # bank at partition bases 0/32/64/96.
        wb = [psum.tile([128, 512], f32, name=f"wb{i}", tag=f"wb{i}")
              for i in range(4)]
        ceb = [psum.tile([128, 512], f32, name=f"ce{i}", tag=f"ce{i}")
               for i in range(4)]

        def w_mm(k, rhs, lhs, start, stop):
            base = 32 * (k % 4)
            nc.tensor.matmul(wb[k // 4][base:base + 1, :], lhs, rhs,
                             start=start, stop=stop, tile_position=(0, base))

        def ce_mm(k, rhs, start, stop):
            base = 32 * (k % 4)
            nc.tensor.matmul(ceb[k // 4][base:base + 1, :], ones[:], rhs,
                             start=start, stop=stop, tile_position=(0, base))

        xtiles, gtiles = {}, {}

        def dma_x(b, nsplit):
            r0 = b * 128
            xb = xpool.tile([128, C], bf16, tag="x", name=f"xb{b}")
            xtiles[b] = xb
            ss = C // nsplit
            for h in range(nsplit):
                sl = slice(h * ss, (h + 1) * ss)
                nc.gpsimd.dma_start(out=xb[:, sl], in_=x_dram[r0:r0 + 128, sl])

        def dma_g(b, nsplit):
            r0 = b * 128
            gb = gpool.tile([128, C], bf16, tag="g", name=f"gb{b}")
            gtiles[b] = gb
            ss = C // nsplit
            for h in range(nsplit):
                sl = slice(h * ss, (h + 1) * ss)
                nc.gpsimd.dma_start(out=gb[:, sl], in_=gt_dram[r0:r0 + 128, sl])

        # startup: block 0's x leads (exp-critical), then its gt
        dma_x(0, 4)
        dma_g(0, 2)

        for b in range(N_BLK):
            xb, gb = xtiles[b], gtiles[b]
            first, last = (b == 0), (b == N_BLK - 1)
            sl0, sl1 = slice(0, F2), slice(F2, C)

            # exp chases block 0's quarter DMAs; all later blocks run
            # halves because the input DMA stream delivers just-in-time
            # (a full-row exp would wait for the whole 4MB x block)
            nsub = 4 if b == 0 else 2
            t = tp.tile([128, C], bf16, tag="t")
            s4 = rowp.tile([128, 8], f32, tag="s4")
            for h in range(nsub):
                ss = C // nsub
                sl = slice(h * ss, (h + 1) * ss)
                nc.scalar.activation(out=t[:, sl], in_=xb[:, sl], func=AF.Exp,
                                     accum_out=s4[:, h:h + 1])
            if nsub > 1:
                st = rowp.tile([128, 1], f32, tag="s")
                nc.vector.tensor_reduce(out=st[:], in_=s4[:, :nsub], op=ALU.add,
                                        axis=mybir.AxisListType.X)
                s_ap = st[:]
            else:
                s_ap = s4[:, 0:1]
            recip = rowp.tile([128, 1], f32, tag="recip")
            nc.vector.reciprocal(out=recip[:], in_=s_ap)
            nc.scalar.activation(out=lse_keep[:, b:b + 1], in_=s_ap, func=AF.Ln)
            # (K - lse) = ln(e^K / s), the lhs for the xp correction stream
            nlK = rowp.tile([128, 1], bf16, tag="nlK")
            nc.scalar.activation(out=nlK[:], in_=recip[:], func=AF.Ln, scale=EK)

            # prime next block's inputs (keeps the SWDGE queue fed);
            # x in halves so the next exp can chase the DMA
            if b + 1 < N_BLK:
                dma_x(b + 1, 2)
                dma_g(b + 1, 1)

            # lngK = ln(g) + K via Ln(e^K * g)
            lng = lngp.tile([128, C], bf16, tag="lng")
            nc.scalar.activation(out=lng[:, sl0], in_=gb[:, sl0], func=AF.Ln,
                                 scale=EK)
            nc.scalar.activation(out=lng[:, sl1], in_=gb[:, sl1], func=AF.Ln,
                                 scale=EK)

            p = pp.tile([128, C], bf16, tag="p")
            nc.vector.tensor_scalar(out=p[:], in0=t[:], scalar1=recip[:],
                                    scalar2=None, op0=ALU.mult)
            for k in range(N_SLICE):
                ksl = slice(k * 512, (k + 1) * 512)
                w_mm(k, p[:, ksl], nlK[:], start=first, stop=False)

            # P3 = p*x (full row) right after p so xb's consumers finish
            # early and the next-next block's x DMA is never buffer-gated
            P3 = prod.tile([128, C], bf16, tag="prod", name=f"p3_{b}")
            nc.vector.tensor_tensor(out=P3[:], in0=p[:], in1=xb[:], op=ALU.mult)
            for k in range(N_SLICE):
                ksl = slice(k * 512, (k + 1) * 512)
                w_mm(k, P3[:, ksl], ones[:], start=False, stop=False)

            u = up.tile([128, C], bf16, tag="u")
            nc.vector.tensor_tensor(out=u[:, sl0], in0=gb[:, sl0], in1=p[:, sl0],
                                    op=ALU.add)
            nc.vector.tensor_tensor(out=u[:, sl1], in0=gb[:, sl1], in1=p[:, sl1],
                                    op=ALU.add)
            # logmK = ln(0.5(u+eps)) + K via Ln(0.5 e^K u + 0.5 e^K eps)
            logm = logmp.tile([128, C], bf16, tag="logm")
            nc.scalar.activation(out=logm[:, sl0], in_=u[:, sl0], func=AF.Ln,
                                 scale=0.5 * EK, bias=eps_b[:])
            nc.scalar.activation(out=logm[:, sl1], in_=u[:, sl1], func=AF.Ln,
                                 scale=0.5 * EK, bias=eps_b[:])

            # P1 = g*lngK (halves: chunk 1's Ln may land late on block 0)
            P1 = prod.tile([128, C], bf16, tag="prod", name=f"p1_{b}")
            for c, slc in ((0, sl0), (1, sl1)):
                nc.vector.tensor_tensor(out=P1[:, slc], in0=gb[:, slc],
                                        in1=lng[:, slc], op=ALU.mult)
                for k in range(c * 8, c * 8 + 8):
                    ksl = slice(k * 512, (k + 1) * 512)
                    w_mm(k, P1[:, ksl], ones[:], start=False, stop=False)

            # P4 = g*x; CE closes before the W group so its PSUM drains
            # while P2 still computes
            P4 = prod.tile([128, C], bf16, tag="prod", name=f"p4_{b}")
            nc.vector.tensor_tensor(out=P4[:], in0=gb[:], in1=xb[:],
                                    op=ALU.mult)
            for k in range(N_SLICE):
                ksl = slice(k * 512, (k + 1) * 512)
                ce_mm(k, P4[:, ksl], start=first, stop=last)

            # P2 last; on the final block split it into quarters so each
            # W PSUM bank closes (and its evac copy starts) progressively
            P2 = prod.tile([128, C], bf16, tag="prod", name=f"p2_{b}")
            np2 = 4 if last else 2
            for c in range(np2):
                cw = C // np2
                slc = slice(c * cw, (c + 1) * cw)
                nc.vector.tensor_tensor(out=P2[:, slc], in0=u[:, slc],
                                        in1=logm[:, slc], op=ALU.mult)
                for k in range(c * (16 // np2), (c + 1) * (16 // np2)):
                    ksl = slice(k * 512, (k + 1) * 512)
                    w_mm(k, P2[:, ksl], negones[:], start=False, stop=last)

        # PSUM is not DMA-readable: bounce through SBUF, then one
        # partition-strided DMA per bank writes its 4 result rows. All
        # copies go on ScalarE (its queue is idle by now, so each bank
        # drains the moment its stop-matmul lands, overlapping the last
        # products still running on VectorE), DMAs on the sync queue.
        for i in range(4):
            sb = consts.tile([128, 512], f32, tag="cebounce", bufs=2)
            nc.scalar.copy(out=sb[:], in_=ceb[i][:])
            nc.sync.dma_start(out=pce_dram[4 * i:4 * i + 4, :],
                              in_=sb[0:128:32, :])
        nc.sync.dma_start(out=lse_dram[:, :], in_=lse_keep[:, :])
        for i in range(4):
            sb = consts.tile([128, 512], f32, tag="wbounce", bufs=2)
            nc.scalar.copy(out=sb[:], in_=wb[i][:])
            nc.sync.dma_start(out=pw_dram[4 * i:4 * i + 4, :],
                              in_=sb[0:128:32, :])

    _split_excess_waits(nc)
    return nc


_NC_CACHE = None
LAST_EXEC_NS = None
LAST_TRACE = None


def kernel(pred_logit: np.ndarray, gt: np.ndarray) -> np.ndarray:
    global _NC_CACHE, LAST_EXEC_NS, LAST_TRACE
    if _NC_CACHE is None:
        _NC_CACHE = build_nc()
    nc = _NC_CACHE

    pred_logit = np.ascontiguousarray(pred_logit, dtype=np.float32)
    gt = np.ascontiguousarray(gt, dtype=np.float32)
    in_maps = [
        {
            "pred_logit": pred_logit[c * ROWS:(c + 1) * ROWS],
            "gt": gt[c * ROWS:(c + 1) * ROWS],
        }
        for c in range(N_CORES)
    ]
    res = run_bass_kernel_spmd(nc, in_maps, list(range(N_CORES)))
    if res.exec_time_ns is not None:
        LAST_EXEC_NS = res.exec_time_ns
        if res.instructions_and_trace:
            LAST_TRACE = res.instructions_and_trace[1]

    w = (C - np.arange(C)).astype(np.float64)
    contrib_total = 0.0
    ce_total = 0.0
    for r in res.results:
        Wc = r["partials_w"].astype(np.float64).reshape(C)
        CEc = r["partials_ce"].astype(np.float64).reshape(C)
        lsec = r["lse_out"].astype(np.float64)
        contrib_total += np.dot(w, Wc)
        ce_total += CEc.sum() - lsec.sum()
    loss = -ce_total / B + 0.25 * contrib_total / B
    return np.array(loss, dtype=np.float32)
